# revision 1
# baseline (speedup 1.0000x reference)
"""Trainium2 Bass kernel for MeanPoolStudyHead (segment-mean + MLP).

Computes, for sorted group_idx:
    means = segment_mean(inst_embs, group_idx, B)        # [B, 1024]
    h     = relu(means @ W1 + b1)                        # [B, 512]
    logits = h @ W2 + b2                                 # [B, 14]

Strategy: data-parallel across 8 NeuronCores, sharded at study boundaries
(chosen to balance instance counts).  Per core, instances are processed in
groups of TB=16 tiles of 128 rows.  For each tile a 0/1 membership matrix
[128 inst, SW study-slots] is built on the vector engine (iota == per-
instance slot id), and the tensor engine accumulates x_tile^T @ mem into
PSUM, producing transposed per-study sums [emb, slot] directly in the
layout the MLP needs.  Studies that straddle a group boundary are assigned
a fixed carry slot (SW-1 in the earlier group, 0 in the later one) and
combined with one small vector add on the scaled means.  1/count scaling
is folded into the PSUM->SBUF copy.  The MLP runs per group on-chip; the
host remaps slot rows to final logits rows.

PSUM banks are shared by several accumulation chains, so each group's
regions are zeroed by K=1 zero-matmuls (start=True resets a whole bank)
and all real matmuls accumulate with start=False.
"""

import math
import os
import sys

sys.path.insert(0, "/opt/trn_rl_repo")

import numpy as np

import concourse.bacc as bacc
import concourse.mybir as mybir
import concourse.tile as tile
from concourse.bass_utils import run_bass_kernel_spmd

NCORES = 8
P = 128          # partitions
TB = 16          # instance tiles per group
SW = 256         # study slots per group window

f32 = mybir.dt.float32
f32r = mybir.dt.float32r

_prog_cache = {}
last_results = None  # stashed BassKernelResults for test harnesses


def _build_program(NT, EMB, HID, NCLS, repeat=1):
    EC = EMB // P    # emb chunks (8)
    HC = HID // P    # hidden chunks (4)
    NG = math.ceil(NT / TB)
    tiles_in = lambda m: TB if m < NG - 1 else NT - (NG - 1) * TB

    nc = bacc.Bacc("TRN2", target_bir_lowering=False, debug=False,
                   num_devices=NCORES)

    x_ext = nc.dram_tensor("x", [NT * P, EMB], f32r, kind="ExternalInput").ap()
    gsh_ext = nc.dram_tensor("gsh", [P, NT], f32, kind="ExternalInput").ap()
    rec_ext = nc.dram_tensor("recipb", [P, NG * SW], f32,
                             kind="ExternalInput").ap()
    iota_ext = nc.dram_tensor("iota", [P, SW], f32, kind="ExternalInput").ap()
    w1_ext = nc.dram_tensor("w1", [EC, P, HID], f32r, kind="ExternalInput").ap()
    w2_ext = nc.dram_tensor("w2", [HC, P, NCLS], f32, kind="ExternalInput").ap()
    b1_ext = nc.dram_tensor("b1t", [P, HC], f32, kind="ExternalInput").ap()
    b2_ext = nc.dram_tensor("b2t", [P, NCLS], f32, kind="ExternalInput").ap()
    out_ext = nc.dram_tensor("logits", [NG * SW, NCLS], f32,
                             kind="ExternalOutput").ap()

    with tile.TileContext(nc) as tc:
        with (
            tc.tile_pool(name="const", bufs=1) as cpool,
            tc.tile_pool(name="xp", bufs=12) as xpool,
            tc.tile_pool(name="mp", bufs=4) as mpool,
            tc.tile_pool(name="mean", bufs=2) as meanpool,
            tc.tile_pool(name="ht", bufs=2) as htpool,
            tc.tile_pool(name="lg", bufs=2) as lgpool,
            tc.tile_pool(name="pseg", bufs=1, space="PSUM") as psegpool,
            tc.tile_pool(name="pht", bufs=1, space="PSUM") as phtpool,
            tc.tile_pool(name="plog", bufs=2, space="PSUM") as plogpool,
        ):
            iota_sb = cpool.tile([P, SW], f32)
            nc.sync.dma_start(iota_sb[:], iota_ext[:])
            gsh_sb = cpool.tile([P, NT], f32)
            nc.sync.dma_start(gsh_sb[:], gsh_ext[:])
            rec_sb = cpool.tile([P, NG * SW], f32)
            nc.sync.dma_start(rec_sb[:], rec_ext[:])
            w1_sb = cpool.tile([P, EC, HID], f32r)
            for e in range(EC):
                nc.sync.dma_start(w1_sb[:, e, :], w1_ext[e])
            w2_sb = cpool.tile([P, HC, NCLS], f32)
            for h in range(HC):
                nc.sync.dma_start(w2_sb[:, h, :], w2_ext[h])
            b1_sb = cpool.tile([P, HC], f32)
            nc.sync.dma_start(b1_sb[:], b1_ext[:])
            b2_sb = cpool.tile([P, NCLS], f32)
            nc.sync.dma_start(b2_sb[:], b2_ext[:])
            # zero operands for PSUM-clearing K=1 matmuls (memset cannot
            # write f32r directly; go through an f32 staging tile)
            ztmp = cpool.tile([1, 2 * SW], f32)
            nc.vector.memset(ztmp[:], 0.0)
            zl_sb = cpool.tile([1, P], f32r)
            nc.vector.tensor_copy(zl_sb[:], ztmp[:, 0:P])
            zr_sb = cpool.tile([1, 2 * SW], f32r)
            nc.vector.tensor_copy(zr_sb[:], ztmp[:])

            def body():
                means_prev = None
                for m in range(NG):
                    ntile = tiles_in(m)
                    # zero pseg banks via K=1 full-bank matmuls
                    pseg = psegpool.tile([P, EC, SW], f32)
                    for b in range(EC // 2):
                        nc.tensor.matmul(
                            pseg[:, 2 * b:2 * b + 2, :], zl_sb[:], zr_sb[:],
                            start=True, stop=True)
                    for t in range(ntile):
                        gt = m * TB + t
                        row0 = gt * P
                        x_t = xpool.tile([P, EMB], f32r)
                        nc.sync.dma_start(x_t[:], x_ext[row0:row0 + P, :])
                        mem = mpool.tile([P, SW], f32r)
                        nc.vector.tensor_scalar(
                            mem[:], iota_sb[:], gsh_sb[:, gt:gt + 1], None,
                            mybir.AluOpType.is_equal)
                        for e in range(EC):
                            nc.tensor.matmul(
                                pseg[:, e, :],
                                x_t[:, e * P:(e + 1) * P],
                                mem[:],
                                start=False, stop=(t == ntile - 1))

                    # scaled means (PSUM * recip -> SBUF), transposed layout
                    means = meanpool.tile([P, EC, SW], f32r)
                    for e in range(EC):
                        nc.vector.tensor_tensor(
                            means[:, e, :], pseg[:, e, :],
                            rec_sb[:, m * SW:(m + 1) * SW],
                            mybir.AluOpType.mult)
                    # fold carry slot from previous group into slot 0
                    if means_prev is not None:
                        for e in range(EC):
                            nc.vector.tensor_tensor(
                                means[:, e, 0:1], means[:, e, 0:1],
                                means_prev[:, e, SW - 1:SW],
                                mybir.AluOpType.add)
                    means_prev = means

                    # h^T = relu(W1^T @ means + b1)
                    pht = phtpool.tile([P, HC, SW], f32)
                    for b in range(HC // 2):
                        nc.tensor.matmul(
                            pht[:, 2 * b:2 * b + 2, :], zl_sb[:], zr_sb[:],
                            start=True, stop=True)
                    for h in range(HC):
                        for e in range(EC):
                            nc.tensor.matmul(
                                pht[:, h, :],
                                w1_sb[:, e, h * P:(h + 1) * P],
                                means[:, e, :],
                                start=False, stop=(e == EC - 1))
                    ht = htpool.tile([P, HC, SW], f32)
                    for h in range(HC):
                        nc.scalar.activation(
                            ht[:, h, :], pht[:, h, :],
                            mybir.ActivationFunctionType.Relu,
                            bias=b1_sb[:, h:h + 1])

                    # logits = h @ W2 + b2, written per 128-slot chunk
                    for sc in range(SW // P):
                        plog = plogpool.tile([P, NCLS], f32)
                        for h in range(HC):
                            nc.tensor.matmul(
                                plog[:],
                                ht[:, h, sc * P:(sc + 1) * P],
                                w2_sb[:, h, :],
                                start=(h == 0), stop=(h == HC - 1))
                        lg = lgpool.tile([P, NCLS], f32)
                        nc.vector.tensor_tensor(
                            lg[:], plog[:], b2_sb[:], mybir.AluOpType.add)
                        nc.sync.dma_start(
                            out_ext[m * SW + sc * P:m * SW + (sc + 1) * P, :],
                            lg[:])

            if repeat > 1:
                with tc.For_i(0, repeat, 1):
                    body()
            else:
                body()

    nc.compile()
    return nc


def _prepare(inst_embs, W1, b1, W2, b2, group_idx, view_idx, batch_size,
             repeat=1):
    x_full = np.ascontiguousarray(np.asarray(inst_embs, dtype=np.float32))
    W1 = np.asarray(W1, dtype=np.float32)
    b1 = np.asarray(b1, dtype=np.float32)
    W2 = np.asarray(W2, dtype=np.float32)
    b2 = np.asarray(b2, dtype=np.float32)
    g = np.asarray(group_idx).astype(np.int64)
    B = int(batch_size)

    N, EMB = x_full.shape
    HID = W1.shape[1]
    NCLS = W2.shape[1]
    assert EMB % P == 0 and HID % P == 0

    counts = np.bincount(g, minlength=B).astype(np.int64)
    assert counts.max() < TB * P, "study larger than one group"
    starts = np.concatenate([[0], np.cumsum(counts)])

    # shard at study boundaries, balancing instance counts
    s_bounds = np.zeros(NCORES + 1, np.int64)
    s_bounds[NCORES] = B
    for k in range(1, NCORES):
        target = k * N // NCORES
        s = int(np.searchsorted(starts, target))
        # starts[s] >= target; compare with previous boundary
        if s > 0 and target - starts[s - 1] < starts[min(s, B)] - target:
            s = s - 1
        s_bounds[k] = min(max(s, s_bounds[k - 1]), B)
    inst_bounds = starts[s_bounds]
    L = np.diff(inst_bounds)
    NT = max(TB, int(math.ceil(L.max() / P)))
    NG = math.ceil(NT / TB)

    key = (NT, EMB, HID, NCLS, repeat)
    if key not in _prog_cache:
        _prog_cache[key] = _build_program(NT, EMB, HID, NCLS, repeat)
    nc = _prog_cache[key]

    # shared tables
    EC, HC = EMB // P, HID // P
    iota_tab = np.broadcast_to(
        np.arange(SW, dtype=np.float32), (P, SW)).copy()
    w1_tab = np.ascontiguousarray(W1.reshape(EC, P, HID))
    w2_tab = np.ascontiguousarray(W2.reshape(HC, P, NCLS))
    b1_tab = np.ascontiguousarray(b1.reshape(HC, P).T)
    b2_tab = np.broadcast_to(b2, (P, NCLS)).copy()

    in_maps = []
    rowmaps = []
    for k in range(NCORES):
        base = int(inst_bounds[k])
        Lk = int(L[k])
        s_lo, s_hi = int(s_bounds[k]), int(s_bounds[k + 1])
        SBk = s_hi - s_lo
        n_rows = NT * P
        end = base + n_rows
        if end <= N:
            xk = x_full[base:end]
        else:
            xk = np.concatenate(
                [x_full[base:], np.zeros((end - N, EMB), np.float32)])

        gl = g[base:base + Lk] - s_lo             # local study ids, sorted
        gshift = np.full(n_rows, -1.0, np.float32)
        recip = np.zeros((NG, SW), np.float32)
        ccounts = counts[s_lo:s_hi]
        cinv = np.where(ccounts > 0,
                        1.0 / np.maximum(ccounts, 1), 0.0).astype(np.float32)

        for m in range(NG):
            lo = m * TB * P
            hi = min((m + 1) * TB * P, NT * P, Lk)
            if lo >= Lk:
                continue
            seg = gl[lo:hi]
            fs = seg[0]
            sl = (seg - fs).astype(np.int64)
            nxt = (m + 1) * TB * P
            if nxt < Lk and m < NG - 1:
                carry_s = gl[nxt]
                is_carry = seg == carry_s
                sl = np.where(is_carry, SW - 1, sl)
                if (~is_carry).any():
                    assert sl[~is_carry].max() <= SW - 2, "window overflow"
            else:
                assert sl.max() <= SW - 2, "window overflow"
            gshift[lo:hi] = sl.astype(np.float32)
            recip[m, sl] = cinv[seg]

        # final row for each local study: slot in the group of its last
        # instance (carry partials flow forward into that group's slot 0)
        rowmap = np.zeros(SBk, np.int64)
        nonempty = np.where(ccounts > 0)[0]
        last_j = (starts[s_lo + nonempty + 1] - 1) - base
        owner = last_j // (TB * P)
        slot = gshift[last_j].astype(np.int64)
        assert (slot >= 0).all() and (slot < SW - 1).all()
        rowmap[nonempty] = owner * SW + slot
        empty = np.where(ccounts == 0)[0]
        if len(empty):
            fm, fsl = np.where(recip[:, 1:SW - 1] == 0)
            assert len(fm) >= len(empty), "no free slots for empty studies"
            rowmap[empty] = fm[:len(empty)] * SW + (fsl[:len(empty)] + 1)
        rowmaps.append(rowmap)

        gsh_tab = np.ascontiguousarray(gshift.reshape(NT, P).T)
        rec_tab = np.broadcast_to(
            recip.reshape(1, NG * SW), (P, NG * SW)).copy()

        in_maps.append({
            "x": np.ascontiguousarray(xk),
            "gsh": gsh_tab,
            "recipb": rec_tab,
            "iota": iota_tab,
            "w1": w1_tab,
            "w2": w2_tab,
            "b1t": b1_tab,
            "b2t": b2_tab,
        })

    return nc, in_maps, rowmaps, (B, s_bounds, NCLS)


def kernel(inst_embs, W1, b1, W2, b2, group_idx, view_idx, batch_size):
    global last_results
    nc, in_maps, rowmaps, (B, s_bounds, NCLS) = _prepare(
        inst_embs, W1, b1, W2, b2, group_idx, view_idx, batch_size)
    res = run_bass_kernel_spmd(nc, in_maps, list(range(NCORES)))
    last_results = res

    out = np.empty((B, NCLS), np.float32)
    for k in range(NCORES):
        out[s_bounds[k]:s_bounds[k + 1]] = res.results[k]["logits"][rowmaps[k]]
    return out


def bench(inputs, iters=5, repeat=1):
    """Time device execution only: inputs pre-staged on device, repeated
    jitted executions, returns (best_seconds, all_times)."""
    nc, in_maps, rowmaps, _ = _prepare(**inputs, repeat=repeat)
    return bench_nc(nc, in_maps, iters)


def bench_nc(nc, in_maps, iters=5):
    """Mirror bass2jax.run_bass_via_pjrt's multi-core path with inputs
    pre-staged on device; time repeated executions."""
    import time

    import jax
    from jax.sharding import Mesh, PartitionSpec, NamedSharding
    from jax.experimental.shard_map import shard_map
    from concourse import bass2jax
    import concourse.mybir as mybir_

    bass2jax.install_neuronx_cc_hook()

    partition_name = (nc.partition_id_tensor.name
                      if nc.partition_id_tensor else None)
    in_names, out_names, out_avals, zero_outs = [], [], [], []
    for alloc in nc.m.functions[0].allocations:
        if not isinstance(alloc, mybir_.MemoryLocationSet):
            continue
        name = alloc.memorylocations[0].name
        if alloc.kind == "ExternalInput":
            if name != partition_name:
                in_names.append(name)
        elif alloc.kind == "ExternalOutput":
            out_names.append(name)
            shape = tuple(alloc.tensor_shape)
            dtype = mybir_.dt.np(alloc.dtype)
            out_avals.append(jax.core.ShapedArray(shape, dtype))
            zero_outs.append(np.zeros(shape, dtype))
    n_params = len(in_names)
    n_outs = len(out_avals)
    all_names = in_names + out_names
    if partition_name is not None:
        all_names.append(partition_name)

    def _body(*args):
        operands = list(args)
        if partition_name is not None:
            operands.append(bass2jax.partition_id_tensor())
        outs = bass2jax._bass_exec_p.bind(
            *operands,
            out_avals=tuple(out_avals),
            in_names=tuple(all_names),
            out_names=tuple(out_names),
            lowering_input_output_aliases=(),
            sim_require_finite=True,
            sim_require_nnan=True,
            nc=nc,
        )
        return tuple(outs)

    devices = jax.devices()[:NCORES]
    mesh = Mesh(np.asarray(devices), ("core",))
    in_specs = (PartitionSpec("core"),) * (n_params + n_outs)
    out_specs = (PartitionSpec("core"),) * n_outs
    sharded = jax.jit(
        shard_map(_body, mesh=mesh, in_specs=in_specs, out_specs=out_specs,
                  check_rep=False),
        keep_unused=True,
    )
    shard = NamedSharding(mesh, PartitionSpec("core"))
    concat_in = [
        jax.device_put(
            np.concatenate([in_maps[c][n] for c in range(NCORES)], axis=0),
            shard)
        for n in in_names
    ]
    concat_zeros = [
        jax.device_put(
            np.zeros((NCORES * z.shape[0], *z.shape[1:]), z.dtype), shard)
        for z in zero_outs
    ]
    times = []
    for _ in range(iters):
        t0 = time.perf_counter()
        out = sharded(*concat_in, *concat_zeros)
        jax.block_until_ready(out)
        times.append(time.perf_counter() - t0)

    # pipelined: launch a burst without blocking, block once at the end
    bursts = []
    for burst in (8, 16):
        out = sharded(*concat_in, *concat_zeros)
        jax.block_until_ready(out)  # warm
        t0 = time.perf_counter()
        outs = [sharded(*concat_in, *concat_zeros) for _ in range(burst)]
        jax.block_until_ready(outs)
        dt = time.perf_counter() - t0
        bursts.append((burst, dt / burst))
    return min(times), (times, bursts)



# revision 2
# speedup vs baseline: 2.2342x; 2.2342x over previous
"""Trainium2 Bass kernel for MeanPoolStudyHead (segment-mean + MLP).

Computes, for sorted group_idx:
    means = segment_mean(inst_embs, group_idx, B)        # [B, 1024]
    h     = relu(means @ W1 + b1)                        # [B, 512]
    logits = h @ W2 + b2                                 # [B, 14]

Strategy: data-parallel across 8 NeuronCores.  The host casts inst_embs to
bf16 (tolerance is 2e-2; bf16 keeps logits error ~1e-3) which halves HBM
traffic, and packs instances into "windows" of up to TB=16 tiles (2048
rows) that each start at a study boundary and contain at most SW=128
studies.  Study slots map to PSUM *partitions*: per 128-row tile a one-hot
membership matrix mem[inst, slot] (built on the vector engine) is the
stationary matmul operand and the x tile streams through as the moving
operand, accumulating pseg[slot, emb] = sum of member embeddings in PSUM.
This costs one small LDWEIGHTS + two N=512 matmuls per tile instead of
eight LDW+MM pairs in the [emb, slot] orientation.

Per window: the scalar engine drains PSUM with a fused per-partition
(1/count) scale into bf16 means[slot, emb]; eight PE transposes flip each
128x128 chunk into meansT[emb, slot]; the MLP runs with W1 chunks
stationary; relu(+b1) on the scalar engine; the logits matmul uses ht
chunks stationary and W2 moving, giving plog[slot, 14] which is written
out per window.  The host maps (window, slot) back to study rows.

PSUM accumulation uses start=True only on the first matmul touching each
bank per window (a start clears the whole 2 KiB bank's has_written bits;
later start=False writes overwrite where clear, accumulate where set), so
no zeroing matmuls are needed.  Post-processing of window m-1 is emitted
after the segment matmuls of window m so the PE never waits on the
scalar/vector engines.
"""

import math
import os
import sys

sys.path.insert(0, "/opt/trn_rl_repo")

import ml_dtypes
import numpy as np

import concourse.bacc as bacc
import concourse.mybir as mybir
import concourse.tile as tile
from concourse.bass_utils import run_bass_kernel_spmd

NCORES = 8
P = 128          # partitions
TB = 16          # instance tiles (of 128 rows) per window
SW = 128         # study slots per window (== PSUM partitions)
ROWS_W = TB * P  # 2048

f32 = mybir.dt.float32
bf16 = mybir.dt.bfloat16
npbf16 = ml_dtypes.bfloat16

_prog_cache = {}
last_results = None  # stashed BassKernelResults for test harnesses


def _build_program(NG, EMB, HID, NCLS, repeat=1):
    EC = EMB // P    # emb chunks (8)
    HC = HID // P    # hidden chunks (4)
    NH = EMB // 2    # matmul moving half-size (512)

    nc = bacc.Bacc("TRN2", target_bir_lowering=False, debug=False,
                   num_devices=NCORES)

    x_ext = nc.dram_tensor("x", [P, NG * TB * EMB], bf16,
                           kind="ExternalInput").ap()
    gsh_ext = nc.dram_tensor("gsh", [P, NG * TB], f32,
                             kind="ExternalInput").ap()
    rec_ext = nc.dram_tensor("recipb", [P, NG], f32,
                             kind="ExternalInput").ap()
    iota_ext = nc.dram_tensor("iota", [P, SW], f32, kind="ExternalInput").ap()
    ident_ext = nc.dram_tensor("ident", [P, P], bf16,
                               kind="ExternalInput").ap()
    w1_ext = nc.dram_tensor("w1", [P, EC * HID], bf16,
                            kind="ExternalInput").ap()
    w2_ext = nc.dram_tensor("w2", [P, HC * NCLS], bf16,
                            kind="ExternalInput").ap()
    b1_ext = nc.dram_tensor("b1t", [P, HC], f32, kind="ExternalInput").ap()
    b2_ext = nc.dram_tensor("b2t", [P, NCLS], f32, kind="ExternalInput").ap()
    out_ext = nc.dram_tensor("logits", [NG * P, NCLS], f32,
                             kind="ExternalOutput").ap()

    with tile.TileContext(nc) as tc:
        with (
            tc.tile_pool(name="const", bufs=1) as cpool,
            tc.tile_pool(name="xp", bufs=3) as xpool,
            tc.tile_pool(name="mp", bufs=4) as mpool,
            tc.tile_pool(name="mean", bufs=2) as meanpool,
            tc.tile_pool(name="mt", bufs=2) as mtpool,
            tc.tile_pool(name="ht", bufs=2) as htpool,
            tc.tile_pool(name="lg", bufs=2) as lgpool,
            tc.tile_pool(name="pseg", bufs=2, space="PSUM") as psegpool,
            tc.tile_pool(name="pmt", bufs=2, space="PSUM") as pmtpool,
            tc.tile_pool(name="pht", bufs=1, space="PSUM") as phtpool,
            tc.tile_pool(name="plog", bufs=1, space="PSUM") as plogpool,
        ):
            iota_sb = cpool.tile([P, SW], f32)
            nc.sync.dma_start(iota_sb[:], iota_ext[:])
            ident_sb = cpool.tile([P, P], bf16)
            nc.sync.dma_start(ident_sb[:], ident_ext[:])
            gsh_sb = cpool.tile([P, NG * TB], f32)
            nc.sync.dma_start(gsh_sb[:], gsh_ext[:])
            rec_sb = cpool.tile([P, NG], f32)
            nc.sync.dma_start(rec_sb[:], rec_ext[:])
            w1_sb = cpool.tile([P, EC * HID], bf16)
            nc.sync.dma_start(w1_sb[:], w1_ext[:])
            w2_sb = cpool.tile([P, HC * NCLS], bf16)
            nc.sync.dma_start(w2_sb[:], w2_ext[:])
            b1_sb = cpool.tile([P, HC], f32)
            nc.sync.dma_start(b1_sb[:], b1_ext[:])
            b2_sb = cpool.tile([P, NCLS], f32)
            nc.sync.dma_start(b2_sb[:], b2_ext[:])

            def post(m, pseg):
                # means[slot, emb] = pseg * (1/count[slot]), drained to bf16
                means = meanpool.tile([P, EC * P], bf16)
                nc.scalar.activation(
                    means[:], pseg[:],
                    mybir.ActivationFunctionType.Copy,
                    scale=rec_sb[:, m:m + 1])
                # meansT[emb, slot] via 8 PE 128x128 transposes
                pmt = pmtpool.tile([P, EC * P], bf16)
                for e in range(EC):
                    nc.tensor.transpose(
                        pmt[:, e * P:(e + 1) * P],
                        means[:, e * P:(e + 1) * P],
                        ident_sb[:])
                mt = mtpool.tile([P, EC * P], bf16)
                nc.vector.tensor_copy(mt[:], pmt[:])
                # h^T[hid, slot] = W1^T @ meansT
                pht = phtpool.tile([P, HC * P], f32)
                for h in range(HC):
                    for e in range(EC):
                        nc.tensor.matmul(
                            pht[:, h * P:(h + 1) * P],
                            w1_sb[:, e * HID + h * P:e * HID + (h + 1) * P],
                            mt[:, e * P:(e + 1) * P],
                            start=(h == 0 and e == 0),
                            stop=(h == HC - 1 and e == EC - 1))
                ht = htpool.tile([P, HC * P], bf16)
                for h in range(HC):
                    nc.scalar.activation(
                        ht[:, h * P:(h + 1) * P], pht[:, h * P:(h + 1) * P],
                        mybir.ActivationFunctionType.Relu,
                        bias=b1_sb[:, h:h + 1])
                # logits[slot, cls] = ht^T @ W2
                plog = plogpool.tile([P, NCLS], f32)
                for h in range(HC):
                    nc.tensor.matmul(
                        plog[:],
                        ht[:, h * P:(h + 1) * P],
                        w2_sb[:, h * NCLS:(h + 1) * NCLS],
                        start=(h == 0), stop=(h == HC - 1))
                lg = lgpool.tile([P, NCLS], f32)
                nc.vector.tensor_tensor(
                    lg[:], plog[:], b2_sb[:], mybir.AluOpType.add)
                nc.sync.dma_start(out_ext[m * P:(m + 1) * P, :], lg[:])

            def body():
                prev = None
                for m in range(NG):
                    x_win = xpool.tile([P, TB * EMB], bf16)
                    nc.sync.dma_start(
                        x_win[:], x_ext[:, m * TB * EMB:(m + 1) * TB * EMB])
                    pseg = psegpool.tile([P, EMB], f32)
                    for t in range(TB):
                        mem = mpool.tile([P, SW], bf16)
                        nc.vector.tensor_scalar(
                            mem[:], iota_sb[:],
                            gsh_sb[:, m * TB + t:m * TB + t + 1], None,
                            mybir.AluOpType.is_equal)
                        for q in range(2):
                            nc.tensor.matmul(
                                pseg[:, q * NH:(q + 1) * NH],
                                mem[:],
                                x_win[:, t * EMB + q * NH:
                                      t * EMB + (q + 1) * NH],
                                start=(t == 0), stop=(t == TB - 1))
                    if prev is not None:
                        post(*prev)
                    prev = (m, pseg)
                post(*prev)

            if repeat > 1:
                with tc.For_i(0, repeat, 1):
                    body()
            else:
                body()

    nc.compile()
    return nc


def _plan_windows(counts, starts):
    """Greedy: consecutive study-id windows, <=SW studies, <=ROWS_W rows."""
    B = len(counts)
    bounds = [0]
    while bounds[-1] < B:
        s0 = bounds[-1]
        # max studies whose rows fit in ROWS_W
        s1 = int(np.searchsorted(starts, starts[s0] + ROWS_W, side="right")) - 1
        s1 = min(s1, s0 + SW, B)
        assert s1 > s0, f"study {s0} larger than one window"
        bounds.append(s1)
    return np.asarray(bounds, np.int64)


def _prepare(inst_embs, W1, b1, W2, b2, group_idx, view_idx, batch_size,
             repeat=1):
    x_full = np.asarray(inst_embs, dtype=np.float32)
    W1 = np.asarray(W1, dtype=np.float32)
    b1 = np.asarray(b1, dtype=np.float32)
    W2 = np.asarray(W2, dtype=np.float32)
    b2 = np.asarray(b2, dtype=np.float32)
    g = np.asarray(group_idx).astype(np.int64)
    B = int(batch_size)

    N, EMB = x_full.shape
    HID = W1.shape[1]
    NCLS = W2.shape[1]
    EC, HC = EMB // P, HID // P
    assert EMB % P == 0 and HID % P == 0

    counts = np.bincount(g, minlength=B).astype(np.int64)
    assert counts.max() <= ROWS_W, "study larger than one window"
    starts = np.concatenate([[0], np.cumsum(counts)])

    wb = _plan_windows(counts, starts)           # study-id window bounds
    W = len(wb) - 1
    NG = math.ceil(W / NCORES)
    # contiguous balanced deal: core k gets windows [wk[k], wk[k+1])
    base, rem = divmod(W, NCORES)
    wk = np.concatenate([[0], np.cumsum(
        [base + (1 if k < rem else 0) for k in range(NCORES)])])

    key = (NG, EMB, HID, NCLS, repeat)
    if key not in _prog_cache:
        _prog_cache[key] = _build_program(NG, EMB, HID, NCLS, repeat)
    nc = _prog_cache[key]

    x_bf = x_full.astype(npbf16)

    iota_tab = np.broadcast_to(
        np.arange(SW, dtype=np.float32), (P, SW)).copy()
    ident_tab = np.eye(P, dtype=npbf16)
    # w1[p, e*HID + j] = W1[e*128+p, j]
    w1_tab = np.ascontiguousarray(
        W1.astype(npbf16).reshape(EC, P, HID).transpose(1, 0, 2)
    ).reshape(P, EC * HID)
    # w2[p, h*NCLS + c] = W2[h*128+p, c]
    w2_tab = np.ascontiguousarray(
        W2.astype(npbf16).reshape(HC, P, NCLS).transpose(1, 0, 2)
    ).reshape(P, HC * NCLS)
    b1_tab = np.ascontiguousarray(b1.reshape(HC, P).T)
    b2_tab = np.broadcast_to(b2, (P, NCLS)).copy()

    in_maps = []
    core_meta = []
    for k in range(NCORES):
        nw = int(wk[k + 1] - wk[k])
        x_core = np.zeros((P, NG * TB * EMB), npbf16)
        gsh = np.full((P, NG * TB), -1.0, np.float32)
        rec = np.zeros((P, NG), np.float32)
        for lm in range(nw):
            s0, s1 = int(wb[wk[k] + lm]), int(wb[wk[k] + lm + 1])
            r0, r1 = int(starts[s0]), int(starts[s1])
            nr = r1 - r0
            xw = np.zeros((ROWS_W, EMB), npbf16)
            xw[:nr] = x_bf[r0:r1]
            x_core[:, lm * TB * EMB:(lm + 1) * TB * EMB] = (
                xw.reshape(TB, P, EMB).transpose(1, 0, 2).reshape(P, TB * EMB))
            slot = np.full(ROWS_W, -1.0, np.float32)
            slot[:nr] = (g[r0:r1] - s0).astype(np.float32)
            gsh[:, lm * TB:(lm + 1) * TB] = slot.reshape(TB, P).T
            c = counts[s0:s1]
            rec[:s1 - s0, lm] = np.where(c > 0, 1.0 / np.maximum(c, 1), 0.0)
        in_maps.append({
            "x": x_core,
            "gsh": gsh,
            "recipb": rec,
            "iota": iota_tab,
            "ident": ident_tab,
            "w1": w1_tab,
            "w2": w2_tab,
            "b1t": b1_tab,
            "b2t": b2_tab,
        })
        core_meta.append((int(wk[k]), nw))

    return nc, in_maps, (B, wb, core_meta, NCLS)


def kernel(inst_embs, W1, b1, W2, b2, group_idx, view_idx, batch_size):
    global last_results
    nc, in_maps, (B, wb, core_meta, NCLS) = _prepare(
        inst_embs, W1, b1, W2, b2, group_idx, view_idx, batch_size)
    res = run_bass_kernel_spmd(nc, in_maps, list(range(NCORES)))
    last_results = res

    out = np.empty((B, NCLS), np.float32)
    for k in range(NCORES):
        w0, nw = core_meta[k]
        logits = res.results[k]["logits"]
        for lm in range(nw):
            s0, s1 = int(wb[w0 + lm]), int(wb[w0 + lm + 1])
            out[s0:s1] = logits[lm * P:lm * P + (s1 - s0)]
    return out


def bench(inputs, iters=5, repeat=1):
    """Time device execution only: inputs pre-staged on device, repeated
    jitted executions, returns (best_seconds, all_times)."""
    nc, in_maps, _ = _prepare(**inputs, repeat=repeat)
    return bench_nc(nc, in_maps, iters)


def bench_nc(nc, in_maps, iters=5):
    """Mirror bass2jax.run_bass_via_pjrt's multi-core path with inputs
    pre-staged on device; time repeated executions."""
    import time

    import jax
    from jax.sharding import Mesh, PartitionSpec, NamedSharding
    from jax.experimental.shard_map import shard_map
    from concourse import bass2jax
    import concourse.mybir as mybir_

    bass2jax.install_neuronx_cc_hook()

    partition_name = (nc.partition_id_tensor.name
                      if nc.partition_id_tensor else None)
    in_names, out_names, out_avals, zero_outs = [], [], [], []
    for alloc in nc.m.functions[0].allocations:
        if not isinstance(alloc, mybir_.MemoryLocationSet):
            continue
        name = alloc.memorylocations[0].name
        if alloc.kind == "ExternalInput":
            if name != partition_name:
                in_names.append(name)
        elif alloc.kind == "ExternalOutput":
            out_names.append(name)
            shape = tuple(alloc.tensor_shape)
            dtype = mybir_.dt.np(alloc.dtype)
            out_avals.append(jax.core.ShapedArray(shape, dtype))
            zero_outs.append(np.zeros(shape, dtype))
    n_params = len(in_names)
    n_outs = len(out_avals)
    all_names = in_names + out_names
    if partition_name is not None:
        all_names.append(partition_name)

    def _body(*args):
        operands = list(args)
        if partition_name is not None:
            operands.append(bass2jax.partition_id_tensor())
        outs = bass2jax._bass_exec_p.bind(
            *operands,
            out_avals=tuple(out_avals),
            in_names=tuple(all_names),
            out_names=tuple(out_names),
            lowering_input_output_aliases=(),
            sim_require_finite=True,
            sim_require_nnan=True,
            nc=nc,
        )
        return tuple(outs)

    devices = jax.devices()[:NCORES]
    mesh = Mesh(np.asarray(devices), ("core",))
    in_specs = (PartitionSpec("core"),) * (n_params + n_outs)
    out_specs = (PartitionSpec("core"),) * n_outs
    sharded = jax.jit(
        shard_map(_body, mesh=mesh, in_specs=in_specs, out_specs=out_specs,
                  check_rep=False),
        keep_unused=True,
    )
    shard = NamedSharding(mesh, PartitionSpec("core"))
    concat_in = [
        jax.device_put(
            np.concatenate([in_maps[c][n] for c in range(NCORES)], axis=0),
            shard)
        for n in in_names
    ]
    concat_zeros = [
        jax.device_put(
            np.zeros((NCORES * z.shape[0], *z.shape[1:]), z.dtype), shard)
        for z in zero_outs
    ]
    times = []
    for _ in range(iters):
        t0 = time.perf_counter()
        out = sharded(*concat_in, *concat_zeros)
        jax.block_until_ready(out)
        times.append(time.perf_counter() - t0)

    # pipelined: launch a burst without blocking, block once at the end
    bursts = []
    for burst in (8, 16):
        out = sharded(*concat_in, *concat_zeros)
        jax.block_until_ready(out)  # warm
        t0 = time.perf_counter()
        outs = [sharded(*concat_in, *concat_zeros) for _ in range(burst)]
        jax.block_until_ready(outs)
        dt = time.perf_counter() - t0
        bursts.append((burst, dt / burst))
    return min(times), (times, bursts)
